# revision 14
# baseline (speedup 1.0000x reference)
"""Trainium2 Bass kernel for nn_DenoiseModule (diffraction removal + 2x2 Wiener).

Math reduction (derived from the reference):
  - The reference FFT2 acts on the (W, C) axes of the (B,H,W,C)-transposed
    image; the Gaussian mask factorizes and the C-axis FFT cancels exactly.
  - Net: per (b,c,h) row, a 1-D circular conv along W with the short kernel
    K = IFFT(s) scaled by a[h], then abs(), then the scipy 2x2 Wiener filter.
  - K's imaginary part contributes <1e-5 rel error -> only the real part is
    convolved, |y| = abs(real conv). The x255/255 scaling cancels; a[h] is
    applied to the input on the host.

Device layout: batch-parallel over 8 cores (4 images = 12 channels each),
W on partitions, fp16 on device (tolerance 2e-2, fp16 lands ~2e-3):
  - conv: 2 fp16 matmuls per 128-row chunk (band block + 16-row halo block)
  - box sums: h-pairs pre-added (um on GPSIMD; us = mag_h^2 + mag_h-1^2 as a
    fused custom DVE op, so mag^2 never materializes), then 2 matmuls per
    chunk per box
  - lvar = ps_bs - ps_lm^2 as a fused custom DVE op with accumulator (noise)
  - Wiener weight w = 1/max(lvar/noise, 1) as a fused custom DVE op
    (seed + 1 Newton-Raphson reciprocal, 8 ALU stages, ~2e-3 rel err)
  - remaining elementwise ops are 4B-aligned fp16 tensor_tensor (2x mode)
"""
import numpy as np

B, C, H, W = 32, 3, 512, 512
NCORES = 8
BL = B // NCORES          # images per core
NCH = BL * C              # channels per core
P = 128
NW = W // P               # w-chunks
TAP = 8                   # conv band half-width
HALO = 16                 # halo rows per chunk (8 below + 8 above)
DR = 40.0

# Chebyshev pair for the bitwise-not reciprocal seed (see dve_ops.py)
_RC0, _RC1 = -0.23549792, 2.0017324


def _constants():
    x_lin = np.linspace(-256, 256, 512).astype(np.float64)
    g = np.exp(-(x_lin ** 2) / (2 * DR * DR))
    sh = (np.arange(512) + 256) % 512
    a = g[sh]                      # per-h scale (fft-order coords)
    K = np.fft.ifft(g[sh])
    d = np.minimum(np.arange(512), 512 - np.arange(512))
    Kr = np.where(d <= TAP, np.real(K), 0.0)
    pp = np.arange(P)[:, None]
    cc = np.arange(P)[None, :]
    # main conv block (chunk-independent): band only, halo covers the rest
    M = np.where(np.abs(cc - pp) <= TAP, Kr[(cc - pp) % 512], 0.0)
    # halo block: rows 0..7 = inputs at chunk_start-8+r, rows 8..15 at +128+(r-8)
    src = np.concatenate([np.arange(-8, 0), np.arange(128, 136)])
    rr_ = src[:, None]
    E = np.where(np.abs(cc - rr_) <= TAP, Kr[(cc - rr_) % 512], 0.0)
    # box lhsT: out[wo] = 0.25*(in[wo] + in[wo-1]); edge: out[0] += 0.25*prev[127]
    bx = np.zeros((P, P))
    np.fill_diagonal(bx, 0.25)
    bx[np.arange(P - 1), np.arange(1, P)] = 0.25
    bxe = np.zeros((P, P))
    bxe[P - 1, 0] = 0.25
    f16 = np.float16
    return (a, M.astype(f16), E.astype(f16), bx.astype(f16), bxe.astype(f16))


_PROG_CACHE = {}


def _install_custom_ops():
    """Register the three fused DVE ops (idempotent)."""
    import concourse.dve_ops as dops
    from concourse.dve_spec import (
        Spec, Src0, Src1, C0, C1, C2, Bin, AluOp, maxx, One, lower,
        _has_src1,
    )
    from concourse.dve_uop import DveOpSpec

    def reg(name, spec):
        for op in dops.OPS:
            if op.name == name:
                return op
        shas = {}
        for ver in ("v3", "v4"):
            tmp = DveOpSpec(name=name, opcode=17,
                            uops=lower(spec, ver=ver), rd1_en=_has_src1(spec))
            shas[ver] = tmp.sha(ver)
        op = dops.DveOp(name, spec, subdim=False, uops_sha=shas)
        dops.OPS.append(op)
        dops.CUSTOM_DVE_SPECS[name] = spec
        dops._SUB_OPCODE_FOR_NAME[name] = 1 + max(
            dops._SUB_OPCODE_FOR_NAME.values()
        )
        return op

    sqsum = reg("SQSUM_ANT", Spec(
        body=Src0 * Src0 + Src1 * Src1,
        reference=lambda in0, in1, s0, s1, imm2: (
            in0.astype(np.float32) ** 2 + in1.astype(np.float32) ** 2
        ).astype(np.float32),
    ))
    lvacc = reg("LVARACC_ANT", Spec(
        body=Src1 - Src0 * Src0,
        accum=AluOp.ADD,
        reference=lambda in0, in1, s0, s1, imm2: (
            in1 - in0.astype(np.float32) ** 2
        ).astype(np.float32),
    ))
    _t = maxx(Src0 * C0, One)
    _nx = Bin(AluOp.BITWISE_NOT, _t, _t)
    _y0 = _nx * C1

    def _ref_w2(in0, in1, s0, s1, imm2):
        t = np.maximum(in0 * s0, 1.0).astype(np.float32)
        nx = (~t.view(np.int32)).view(np.float32)
        y0 = nx * s1
        return (y0 * (imm2 - t * y0) * in1).astype(np.float32)

    # w1 = diff / max(lvar/noise, 1): seed + 1 Newton-Raphson reciprocal of
    # the clamped ratio, multiplied by diff — all in one 8-stage DVE pass
    wien = reg("WIENW2_ANT", Spec(
        body=_y0 * (C2 - _t * _y0) * Src1, reference=_ref_w2,
    ))
    return sqsum, lvacc, wien


def _build_program():
    from contextlib import ExitStack
    import concourse.bacc as bacc
    import concourse.tile as tile
    from concourse import mybir

    f32 = mybir.dt.float32
    f16 = mybir.dt.float16
    Alu = mybir.AluOpType
    Act = mybir.ActivationFunctionType

    sqsum_op, lvacc_op, wien_op = _install_custom_ops()

    nc = bacc.Bacc(None)
    x_in = nc.declare_dram_parameter("x", [NCH, W, H], f16, isOutput=False)
    xh_in = nc.declare_dram_parameter("xh", [NCH, HALO, NW, H], f16, isOutput=False)
    cm_in = nc.declare_dram_parameter("cm", [P, P], f16, isOutput=False)
    ce_in = nc.declare_dram_parameter("ce", [HALO, P], f16, isOutput=False)
    bx_in = nc.declare_dram_parameter("bx", [P, P], f16, isOutput=False)
    bxe_in = nc.declare_dram_parameter("bxe", [P, P], f16, isOutput=False)
    ones_in = nc.declare_dram_parameter("ones", [P, 1], f32, isOutput=False)
    onesr_in = nc.declare_dram_parameter("onesr", [1, P], f32, isOutput=False)
    y_out = nc.declare_dram_parameter("y", [NCH, W, H], f16, isOutput=True)

    HP = H + 2

    with tile.TileContext(nc) as tc, ExitStack() as ctx:
        cpool = ctx.enter_context(tc.tile_pool(name="consts", bufs=1))
        cm_t = cpool.tile([P, P], f16, tag="cm")
        nc.sync.dma_start(cm_t[:], cm_in[:])
        ce_t = cpool.tile([HALO, P], f16, tag="ce")
        nc.sync.dma_start(ce_t[:], ce_in[:])
        bx_t = cpool.tile([P, P], f16, tag="bx")
        nc.sync.dma_start(bx_t[:], bx_in[:])
        bxe_t = cpool.tile([P, P], f16, tag="bxe")
        nc.sync.dma_start(bxe_t[:], bxe_in[:])
        ones_t = cpool.tile([P, 1], f32, tag="ones")
        nc.sync.dma_start(ones_t[:], ones_in[:])
        onesr_t = cpool.tile([1, P], f32, tag="onesr")
        nc.sync.dma_start(onesr_t[:], onesr_in[:])

        xpool = ctx.enter_context(tc.tile_pool(name="xin", bufs=3))
        hpool = ctx.enter_context(tc.tile_pool(name="halo", bufs=3))
        mpool = ctx.enter_context(tc.tile_pool(name="magp", bufs=3))
        upool = ctx.enter_context(tc.tile_pool(name="up", bufs=3))
        lpool = ctx.enter_context(tc.tile_pool(name="lvp", bufs=2))
        tpool = ctx.enter_context(tc.tile_pool(name="tmp", bufs=3))
        npool = ctx.enter_context(tc.tile_pool(name="noise", bufs=3))
        opool = ctx.enter_context(tc.tile_pool(name="outp", bufs=2))
        psum = ctx.enter_context(tc.tile_pool(name="ps", bufs=2, space="PSUM"))

        def emit_pass_a(ch):
            st = {"ch": ch}
            xin = xpool.tile([P, NW, H], f16, tag="xin")
            nc.sync.dma_start(
                xin[:], x_in[ch].rearrange("(j p) h -> p j h", p=P)
            )
            halo = hpool.tile([HALO, NW, H], f16, tag="halo")
            nc.sync.dma_start(halo[:], xh_in[ch])

            # mag keeps 2 leading zero cols per chunk: data at [2:H+2) so the
            # h-1 shifted read [1:H+1) sees a zero at h=0.
            mag = mpool.tile([P, NW, HP], f16, tag="mag")
            nc.vector.memset(mag[:, :, 0:2], 0.0)
            # second copy of mag (filled per-chunk below via SBUF->SBUF DMA)
            # so two-tensor DVE reads of (mag_h, mag_h-1) hit different tiles
            # instead of double-reading one tile (SBUF port conflict)
            magb = mpool.tile([P, NW, HP], f16, tag="magb")
            nc.vector.memset(magb[:, :, 0:2], 0.0)
            lvar = lpool.tile([P, NW, H], f16, tag="lvar")
            diff = lpool.tile([P, NW, H], f16, tag="diff")
            part = tpool.tile([P, NW], f32, tag="part")
            um_prev = None
            us_prev = None

            for i in range(NW):
                ps_y = psum.tile([P, H], f32, tag="ps_y")
                nc.tensor.matmul(ps_y[:], cm_t[:], xin[:, i, :],
                                 start=True, stop=False)
                nc.tensor.matmul(ps_y[:], ce_t[:], halo[:, i, :],
                                 start=False, stop=True)
                nc.scalar.activation(mag[:, i, 2:H + 2], ps_y[:], Act.Abs)
                nc.sync.dma_start(magb[:, i, 2:H + 2], mag[:, i, 1:H + 1])
                # h-pair pre-adds: um on GPSIMD, us fused (mag^2 + mag_h-1^2)
                um = upool.tile([P, H], f16, tag="um")
                nc.gpsimd.tensor_tensor(
                    um[:], mag[:, i, 2:H + 2], magb[:, i, 2:H + 2], Alu.add
                )
                us = upool.tile([P, H], f16, tag="us")
                nc.vector._custom_dve(
                    sqsum_op, out=us[:], in0=mag[:, i, 2:H + 2],
                    in1=magb[:, i, 2:H + 2],
                )
                # box sums on PE: ps_lm = box(mag)/4, ps_bs = box(mag^2)/4
                ps_lm = psum.tile([P, H], f32, tag="ps_lm")
                nc.tensor.matmul(ps_lm[:], bx_t[:], um[:],
                                 start=True, stop=(i == 0))
                if i > 0:
                    nc.tensor.matmul(ps_lm[:], bxe_t[:], um_prev[:],
                                     start=False, stop=True)
                ps_bs = psum.tile([P, H], f32, tag="ps_bs")
                nc.tensor.matmul(ps_bs[:], bx_t[:], us[:],
                                 start=True, stop=(i == 0))
                if i > 0:
                    nc.tensor.matmul(ps_bs[:], bxe_t[:], us_prev[:],
                                     start=False, stop=True)
                um_prev, us_prev = um, us

                # lM to SBUF (fp16) for diff; lvar fused from the two psums
                lm = tpool.tile([P, H], f16, tag="lm")
                nc.scalar.copy(lm[:], ps_lm[:])
                nc.vector._custom_dve(
                    lvacc_op, out=lvar[:, i, :], in0=lm[:], in1=ps_bs[:],
                    accum_out=part[:, i:i + 1],
                )
                # diff = lM - mag (all fp16, 4B-aligned -> 2x mode)
                nc.vector.tensor_tensor(diff[:, i, :], lm[:],
                                        mag[:, i, 2:H + 2], Alu.subtract)

            # ---- noise scalar (PE reduce + broadcast) ----
            pr = tpool.tile([P, 1], f32, tag="pr")
            nc.vector.tensor_reduce(pr[:], part[:], mybir.AxisListType.X, Alu.add)
            ps_n1 = psum.tile([P, H], f32, tag="ps_y")
            nc.tensor.matmul(ps_n1[:1, :1], ones_t[:], pr[:], start=True, stop=True)
            nb = npool.tile([1, 1], f32, tag="nb")
            nc.scalar.copy(nb[:], ps_n1[:1, :1])
            ps_n2 = psum.tile([P, H], f32, tag="ps_lm")
            nc.tensor.matmul(ps_n2[:, :1], onesr_t[:], nb[:],
                             start=True, stop=True)
            noise = npool.tile([P, 1], f32, tag="noise")
            nc.scalar.mul(noise[:], ps_n2[:, :1], 1.0 / (H * W))
            inv_noise = npool.tile([P, 1], f32, tag="inv_noise")
            nc.vector.reciprocal_approx_fast(inv_noise[:], noise[:])
            st.update(mag=mag, lvar=lvar, diff=diff, inv_noise=inv_noise)
            return st

        def emit_pass_b(st):
            ch = st["ch"]
            mag, lvar, diff = st["mag"], st["lvar"], st["diff"]
            inv_noise = st["inv_noise"]
            # w1 = diff / max(lvar/noise, 1)  (fused seed+1NR recip * diff);
            # per-chunk: the TTSS struct needs in1 with a single free dim
            w1 = tpool.tile([P, NW, H], f16, tag="w1")
            for i in range(NW):
                nc.vector._custom_dve(
                    wien_op, out=w1[:, i, :], in0=lvar[:, i, :],
                    in1=diff[:, i, :],
                    s0=inv_noise[:], s1=_RC0, imm2=_RC1,
                )
            # out = mag + w1 (aligned fp16 TT -> 2x)
            out_t = opool.tile([P, NW, H], f16, tag="out")
            nc.vector.tensor_tensor(
                out_t[:], mag[:, :, 2:H + 2], w1[:], Alu.add
            )
            nc.scalar.dma_start(
                y_out[ch].rearrange("(j p) h -> p j h", p=P), out_t[:]
            )

        prev = None
        for ch in range(NCH):
            st = emit_pass_a(ch)
            if prev is not None:
                emit_pass_b(prev)
            prev = st
        emit_pass_b(prev)

    nc.finalize()
    return nc


def _get_prog():
    if "prog" not in _PROG_CACHE:
        a, M, E, bx, bxe = _constants()
        _PROG_CACHE["a"] = a
        _PROG_CACHE["cm"] = M
        _PROG_CACHE["ce"] = E
        _PROG_CACHE["bx"] = bx
        _PROG_CACHE["bxe"] = bxe
        _PROG_CACHE["prog"] = _build_program()
    return _PROG_CACHE["prog"]


def _run(image, **spmd_kwargs):
    from concourse.bass_utils import run_bass_kernel_spmd

    nc = _get_prog()
    a = _PROG_CACHE["a"]
    # host prep: transpose to (b,c,w,h), scale by a[h], cast fp16
    xt = np.transpose(np.asarray(image, np.float64), (0, 1, 3, 2)) * a[None, None, None, :]
    xt16 = np.ascontiguousarray(xt.astype(np.float16)).reshape(NCORES, NCH, W, H)
    # halo rows per chunk: src rows (i*128 + {-8..-1, 128..135}) mod 512
    src = np.concatenate([np.arange(-8, 0), np.arange(128, 136)])  # (16,)
    rows = (np.arange(NW)[None, :] * P + src[:, None]) % W          # (16, NW)
    xh = np.ascontiguousarray(xt16[:, :, rows, :])                  # (8, NCH, 16, NW, H)
    consts = {k: _PROG_CACHE[k] for k in ("cm", "ce", "bx", "bxe")}
    consts["ones"] = np.ones((P, 1), np.float32)
    consts["onesr"] = np.ones((1, P), np.float32)
    in_maps = [{"x": xt16[c], "xh": xh[c], **consts} for c in range(NCORES)]
    res = run_bass_kernel_spmd(nc, in_maps, list(range(NCORES)), **spmd_kwargs)
    ys = np.stack([res.results[c]["y"] for c in range(NCORES)])  # (8, 12, W, H) f16
    out = ys.reshape(B, C, W, H).transpose(0, 1, 3, 2).astype(np.float32)
    return np.ascontiguousarray(out), res


def kernel(image):
    out, _ = _run(image)
    return out


# revision 16
# speedup vs baseline: 1.1638x; 1.1638x over previous
"""Trainium2 Bass kernel for nn_DenoiseModule (diffraction removal + 2x2 Wiener).

Math reduction (derived from the reference):
  - The reference FFT2 acts on the (W, C) axes of the (B,H,W,C)-transposed
    image; the Gaussian mask factorizes and the C-axis FFT cancels exactly.
  - Net: per (b,c,h) row, a 1-D circular conv along W with the short kernel
    K = IFFT(s) scaled by a[h], then abs(), then the scipy 2x2 Wiener filter.
  - K's imaginary part contributes <1e-5 rel error -> only the real part is
    convolved, |y| = abs(real conv). The x255/255 scaling cancels; a[h] is
    applied to the input on the host.

Device layout: batch-parallel over 8 cores (4 images = 12 channels each),
W on partitions, fp16 on device (tolerance 2e-2, fp16 lands ~2e-3):
  - conv: 2 fp16 matmuls per 128-row chunk (band block + 16-row halo block)
  - box sums: h-pairs pre-added (um on GPSIMD; us = mag_h^2 + mag_h-1^2 as a
    fused custom DVE op, so mag^2 never materializes), then 2 matmuls per
    chunk per box
  - lvar = ps_bs - ps_lm^2 as a fused custom DVE op with accumulator (noise)
  - Wiener weight w = 1/max(lvar/noise, 1) as a fused custom DVE op
    (seed + 1 Newton-Raphson reciprocal, 8 ALU stages, ~2e-3 rel err)
  - remaining elementwise ops are 4B-aligned fp16 tensor_tensor (2x mode)
"""
import numpy as np

B, C, H, W = 32, 3, 512, 512
NCORES = 8
BL = B // NCORES          # images per core
NCH = BL * C              # channels per core
P = 128
NW = W // P               # w-chunks
TAP = 8                   # conv band half-width
HALO = 16                 # halo rows per chunk (8 below + 8 above)
DR = 40.0

# Chebyshev pair for the bitwise-not reciprocal seed (see dve_ops.py)
_RC0, _RC1 = -0.23549792, 2.0017324


def _constants():
    x_lin = np.linspace(-256, 256, 512).astype(np.float64)
    g = np.exp(-(x_lin ** 2) / (2 * DR * DR))
    sh = (np.arange(512) + 256) % 512
    a = g[sh]                      # per-h scale (fft-order coords)
    K = np.fft.ifft(g[sh])
    d = np.minimum(np.arange(512), 512 - np.arange(512))
    Kr = np.where(d <= TAP, np.real(K), 0.0)
    pp = np.arange(P)[:, None]
    cc = np.arange(P)[None, :]
    # main conv block (chunk-independent): band only, halo covers the rest
    M = np.where(np.abs(cc - pp) <= TAP, Kr[(cc - pp) % 512], 0.0)
    # halo block: rows 0..7 = inputs at chunk_start-8+r, rows 8..15 at +128+(r-8)
    src = np.concatenate([np.arange(-8, 0), np.arange(128, 136)])
    rr_ = src[:, None]
    E = np.where(np.abs(cc - rr_) <= TAP, Kr[(cc - rr_) % 512], 0.0)
    # box lhsT: out[wo] = 0.25*(in[wo] + in[wo-1]); edge: out[0] += 0.25*prev[127]
    bx = np.zeros((P, P))
    np.fill_diagonal(bx, 0.25)
    bx[np.arange(P - 1), np.arange(1, P)] = 0.25
    bxe = np.zeros((P, P))
    bxe[P - 1, 0] = 0.25
    f16 = np.float16
    return (a, M.astype(f16), E.astype(f16), bx.astype(f16), bxe.astype(f16))


_PROG_CACHE = {}


def _install_custom_ops():
    """Register the three fused DVE ops (idempotent)."""
    import concourse.dve_ops as dops
    from concourse.dve_spec import (
        Spec, Src0, Src1, C0, C1, C2, Bin, AluOp, maxx, One, lower,
        _has_src1,
    )
    from concourse.dve_uop import DveOpSpec

    def reg(name, spec):
        for op in dops.OPS:
            if op.name == name:
                return op
        shas = {}
        for ver in ("v3", "v4"):
            tmp = DveOpSpec(name=name, opcode=17,
                            uops=lower(spec, ver=ver), rd1_en=_has_src1(spec))
            shas[ver] = tmp.sha(ver)
        op = dops.DveOp(name, spec, subdim=False, uops_sha=shas)
        dops.OPS.append(op)
        dops.CUSTOM_DVE_SPECS[name] = spec
        dops._SUB_OPCODE_FOR_NAME[name] = 1 + max(
            dops._SUB_OPCODE_FOR_NAME.values()
        )
        return op

    sqsum = reg("SQSUM_ANT", Spec(
        body=Src0 * Src0 + Src1 * Src1,
        reference=lambda in0, in1, s0, s1, imm2: (
            in0.astype(np.float32) ** 2 + in1.astype(np.float32) ** 2
        ).astype(np.float32),
    ))
    lvacc = reg("LVARACC_ANT", Spec(
        body=Src1 - Src0 * Src0,
        accum=AluOp.ADD,
        reference=lambda in0, in1, s0, s1, imm2: (
            in1 - in0.astype(np.float32) ** 2
        ).astype(np.float32),
    ))
    _t = maxx(Src0 * C0, One)
    _nx = Bin(AluOp.BITWISE_NOT, _t, _t)
    _y0 = _nx * C1

    def _ref_w2(in0, in1, s0, s1, imm2):
        t = np.maximum(in0 * s0, 1.0).astype(np.float32)
        nx = (~t.view(np.int32)).view(np.float32)
        y0 = nx * s1
        return (y0 * (imm2 - t * y0) * in1).astype(np.float32)

    # w1 = diff / max(lvar/noise, 1): seed + 1 Newton-Raphson reciprocal of
    # the clamped ratio, multiplied by diff — all in one 8-stage DVE pass
    wien = reg("WIENW2_ANT", Spec(
        body=_y0 * (C2 - _t * _y0) * Src1, reference=_ref_w2,
    ))
    return sqsum, lvacc, wien


def _build_program():
    from contextlib import ExitStack
    import concourse.bacc as bacc
    import concourse.tile as tile
    from concourse import mybir

    f32 = mybir.dt.float32
    f16 = mybir.dt.float16
    Alu = mybir.AluOpType
    Act = mybir.ActivationFunctionType

    sqsum_op, lvacc_op, wien_op = _install_custom_ops()

    nc = bacc.Bacc(None)
    x_in = nc.declare_dram_parameter("x", [NCH, W, H], f16, isOutput=False)
    xh_in = nc.declare_dram_parameter("xh", [NCH, HALO, NW, H], f16, isOutput=False)
    cm_in = nc.declare_dram_parameter("cm", [P, P], f16, isOutput=False)
    ce_in = nc.declare_dram_parameter("ce", [HALO, P], f16, isOutput=False)
    bx_in = nc.declare_dram_parameter("bx", [P, P], f16, isOutput=False)
    bxe_in = nc.declare_dram_parameter("bxe", [P, P], f16, isOutput=False)
    ones_in = nc.declare_dram_parameter("ones", [P, 1], f32, isOutput=False)
    onesr_in = nc.declare_dram_parameter("onesr", [1, P], f32, isOutput=False)
    y_out = nc.declare_dram_parameter("y", [NCH, W, H], f16, isOutput=True)

    HP = H + 2

    with tile.TileContext(nc) as tc, ExitStack() as ctx:
        cpool = ctx.enter_context(tc.tile_pool(name="consts", bufs=1))
        cm_t = cpool.tile([P, P], f16, tag="cm")
        nc.sync.dma_start(cm_t[:], cm_in[:])
        ce_t = cpool.tile([HALO, P], f16, tag="ce")
        nc.sync.dma_start(ce_t[:], ce_in[:])
        bx_t = cpool.tile([P, P], f16, tag="bx")
        nc.sync.dma_start(bx_t[:], bx_in[:])
        bxe_t = cpool.tile([P, P], f16, tag="bxe")
        nc.sync.dma_start(bxe_t[:], bxe_in[:])
        ones_t = cpool.tile([P, 1], f32, tag="ones")
        nc.sync.dma_start(ones_t[:], ones_in[:])
        onesr_t = cpool.tile([1, P], f32, tag="onesr")
        nc.sync.dma_start(onesr_t[:], onesr_in[:])

        xpool = ctx.enter_context(tc.tile_pool(name="xin", bufs=3))
        hpool = ctx.enter_context(tc.tile_pool(name="halo", bufs=3))
        mpool = ctx.enter_context(tc.tile_pool(name="magp", bufs=3))
        upool = ctx.enter_context(tc.tile_pool(name="up", bufs=3))
        lpool = ctx.enter_context(tc.tile_pool(name="lvp", bufs=2))
        tpool = ctx.enter_context(tc.tile_pool(name="tmp", bufs=3))
        npool = ctx.enter_context(tc.tile_pool(name="noise", bufs=3))
        opool = ctx.enter_context(tc.tile_pool(name="outp", bufs=2))
        psum = ctx.enter_context(tc.tile_pool(name="ps", bufs=2, space="PSUM"))

        def emit_pass_a(ch):
            st = {"ch": ch}
            xin = xpool.tile([P, NW, H], f16, tag="xin")
            nc.sync.dma_start(
                xin[:], x_in[ch].rearrange("(j p) h -> p j h", p=P)
            )
            halo = hpool.tile([HALO, NW, H], f16, tag="halo")
            nc.sync.dma_start(halo[:], xh_in[ch])

            # mag keeps 2 leading zero cols per chunk: data at [2:H+2) so the
            # h-1 shifted read [1:H+1) sees a zero at h=0.
            mag = mpool.tile([P, NW, HP], f16, tag="mag")
            nc.vector.memset(mag[:, :, 0:2], 0.0)
            # second copy of mag (filled per-chunk below via SBUF->SBUF DMA)
            # so two-tensor DVE reads of (mag_h, mag_h-1) hit different tiles
            # instead of double-reading one tile (SBUF port conflict)
            magb = mpool.tile([P, NW, HP], f16, tag="magb")
            nc.vector.memset(magb[:, :, 0:2], 0.0)
            lvar = lpool.tile([P, NW, H], f16, tag="lvar")
            diff = lpool.tile([P, NW, H], f16, tag="diff")
            part = tpool.tile([P, NW], f32, tag="part")
            us_prev = None

            for i in range(NW):
                ps_y = psum.tile([P, H], f32, tag="ps_y")
                nc.tensor.matmul(ps_y[:], cm_t[:], xin[:, i, :],
                                 start=True, stop=False)
                nc.tensor.matmul(ps_y[:], ce_t[:], halo[:, i, :],
                                 start=False, stop=True)
                nc.scalar.activation(mag[:, i, 2:H + 2], ps_y[:], Act.Abs)
                nc.sync.dma_start(magb[:, i, 2:H + 2], mag[:, i, 1:H + 1])
                # us = mag_h^2 + mag_h-1^2 fused (both reads 4B-aligned)
                us = upool.tile([P, H], f16, tag="us")
                nc.vector._custom_dve(
                    sqsum_op, out=us[:], in0=mag[:, i, 2:H + 2],
                    in1=magb[:, i, 2:H + 2],
                )
                # box sums on PE; lm box h-pumped from mag + shifted copy
                ps_lm = psum.tile([P, H], f32, tag="ps_lm")
                nc.tensor.matmul(ps_lm[:], bx_t[:], mag[:, i, 2:H + 2],
                                 start=True, stop=False)
                nc.tensor.matmul(ps_lm[:], bx_t[:], magb[:, i, 2:H + 2],
                                 start=False, stop=(i == 0))
                if i > 0:
                    nc.tensor.matmul(ps_lm[:], bxe_t[:],
                                     mag[:, i - 1, 2:H + 2],
                                     start=False, stop=False)
                    nc.tensor.matmul(ps_lm[:], bxe_t[:],
                                     magb[:, i - 1, 2:H + 2],
                                     start=False, stop=True)
                ps_bs = psum.tile([P, H], f32, tag="ps_bs")
                nc.tensor.matmul(ps_bs[:], bx_t[:], us[:],
                                 start=True, stop=(i == 0))
                if i > 0:
                    nc.tensor.matmul(ps_bs[:], bxe_t[:], us_prev[:],
                                     start=False, stop=True)
                us_prev = us

                # lM to SBUF (fp16) for diff; lvar fused from the two psums
                lm = tpool.tile([P, H], f16, tag="lm")
                nc.scalar.copy(lm[:], ps_lm[:])
                nc.vector._custom_dve(
                    lvacc_op, out=lvar[:, i, :], in0=lm[:], in1=ps_bs[:],
                    accum_out=part[:, i:i + 1],
                )
                # diff = lM - mag (all fp16, 4B-aligned -> 2x mode)
                nc.vector.tensor_tensor(diff[:, i, :], lm[:],
                                        mag[:, i, 2:H + 2], Alu.subtract)

            # ---- noise scalar (PE reduce + broadcast) ----
            pr = tpool.tile([P, 1], f32, tag="pr")
            nc.vector.tensor_reduce(pr[:], part[:], mybir.AxisListType.X, Alu.add)
            ps_n1 = psum.tile([P, H], f32, tag="ps_y")
            nc.tensor.matmul(ps_n1[:1, :1], ones_t[:], pr[:], start=True, stop=True)
            nb = npool.tile([1, 1], f32, tag="nb")
            nc.scalar.copy(nb[:], ps_n1[:1, :1])
            ps_n2 = psum.tile([P, H], f32, tag="ps_lm")
            nc.tensor.matmul(ps_n2[:, :1], onesr_t[:], nb[:],
                             start=True, stop=True)
            noise = npool.tile([P, 1], f32, tag="noise")
            nc.scalar.mul(noise[:], ps_n2[:, :1], 1.0 / (H * W))
            inv_noise = npool.tile([P, 1], f32, tag="inv_noise")
            nc.vector.reciprocal_approx_fast(inv_noise[:], noise[:])
            st.update(mag=mag, lvar=lvar, diff=diff, inv_noise=inv_noise)
            return st

        def emit_pass_b(st):
            ch = st["ch"]
            mag, lvar, diff = st["mag"], st["lvar"], st["diff"]
            inv_noise = st["inv_noise"]
            # w1 = diff / max(lvar/noise, 1)  (fused seed+1NR recip * diff);
            # per-chunk: the TTSS struct needs in1 with a single free dim
            w1 = tpool.tile([P, NW, H], f16, tag="w1")
            for i in range(NW):
                nc.vector._custom_dve(
                    wien_op, out=w1[:, i, :], in0=lvar[:, i, :],
                    in1=diff[:, i, :],
                    s0=inv_noise[:], s1=_RC0, imm2=_RC1,
                )
            # out = mag + w1 (aligned fp16 TT -> 2x)
            out_t = opool.tile([P, NW, H], f16, tag="out")
            nc.vector.tensor_tensor(
                out_t[:], mag[:, :, 2:H + 2], w1[:], Alu.add
            )
            nc.scalar.dma_start(
                y_out[ch].rearrange("(j p) h -> p j h", p=P), out_t[:]
            )

        prev = None
        for ch in range(NCH):
            st = emit_pass_a(ch)
            if prev is not None:
                emit_pass_b(prev)
            prev = st
        emit_pass_b(prev)

    nc.finalize()
    return nc


def _get_prog():
    if "prog" not in _PROG_CACHE:
        a, M, E, bx, bxe = _constants()
        _PROG_CACHE["a"] = a
        _PROG_CACHE["cm"] = M
        _PROG_CACHE["ce"] = E
        _PROG_CACHE["bx"] = bx
        _PROG_CACHE["bxe"] = bxe
        _PROG_CACHE["prog"] = _build_program()
    return _PROG_CACHE["prog"]


def _run(image, **spmd_kwargs):
    from concourse.bass_utils import run_bass_kernel_spmd

    nc = _get_prog()
    a = _PROG_CACHE["a"]
    # host prep: transpose to (b,c,w,h), scale by a[h], cast fp16
    xt = np.transpose(np.asarray(image, np.float64), (0, 1, 3, 2)) * a[None, None, None, :]
    xt16 = np.ascontiguousarray(xt.astype(np.float16)).reshape(NCORES, NCH, W, H)
    # halo rows per chunk: src rows (i*128 + {-8..-1, 128..135}) mod 512
    src = np.concatenate([np.arange(-8, 0), np.arange(128, 136)])  # (16,)
    rows = (np.arange(NW)[None, :] * P + src[:, None]) % W          # (16, NW)
    xh = np.ascontiguousarray(xt16[:, :, rows, :])                  # (8, NCH, 16, NW, H)
    consts = {k: _PROG_CACHE[k] for k in ("cm", "ce", "bx", "bxe")}
    consts["ones"] = np.ones((P, 1), np.float32)
    consts["onesr"] = np.ones((1, P), np.float32)
    in_maps = [{"x": xt16[c], "xh": xh[c], **consts} for c in range(NCORES)]
    res = run_bass_kernel_spmd(nc, in_maps, list(range(NCORES)), **spmd_kwargs)
    ys = np.stack([res.results[c]["y"] for c in range(NCORES)])  # (8, 12, W, H) f16
    out = ys.reshape(B, C, W, H).transpose(0, 1, 3, 2).astype(np.float32)
    return np.ascontiguousarray(out), res


def kernel(image):
    out, _ = _run(image)
    return out
